# revision 1
# baseline (speedup 1.0000x reference)
"""Trainium2 Bass kernel for nn_Metric_35588099015209.

Reference computation (BS=64, IN_DIM=9801, OUT_DIM=25):
    d      = einsum('oi,bi->bo', measure,     input)   # (64, 25)
    d_full = einsum('fi,bi->bf', fullmeasure, input)   # (64, 9801)

Structure exploited (guaranteed by the Metric module's construction and
verified at runtime): `measure` is 25 distinct rows of the identity, and
`fullmeasure` is the identity with the complementary rows zeroed — i.e. a
diagonal 0/1 matrix whose 25 ones sit exactly at the columns `measure`
selects. Therefore:
    d[b, o]      = input[b, idx[o]]              (a 25-column gather)
    d_full[b, f] = input[b, f] if f in idx else 0 (a sparse column mask)

Sharding: data-parallel over the batch — each of the 8 NeuronCores gets 8
rows of `input` (passed transposed, (9801, 8), so one live feature is a
contiguous 32-byte row) and produces its 8 rows of both outputs. No
communication. On-core, the d_full image is built in a (128, 616) SBUF tile
(partition = chunk*8 + batch_row, 16 chunks of <=616 features cover 9801):
memset zeros, DMA-poke the 25 live columns, DMA the image out per chunk.
The compacted d tile is gathered by 25 tiny column DMAs.
"""

import numpy as np

IN_DIM = 9801
OUT_DIM = 25
BS = 64
N_CORES = 8
BPC = BS // N_CORES  # batch rows per core
NCHUNK = 16
CHUNK = 616  # 15*616 + 561 = 9801
LAST = IN_DIM - (NCHUNK - 1) * CHUNK  # 561


def _install_drain_patch():
    """This container's walrus build rejects the multi-wait Drain that
    TileContext emits at kernel end ('Too many sync wait commands'). Split
    it into a chain of single-wait Drains — semantically identical."""
    import concourse.tile as tile
    import concourse.mybir as mybir
    from concourse.vector_clock import ScopedClock

    if getattr(tile.TileContext, "_drain_patch_installed", False):
        return

    def _split_drain_and_barrier(self, tick_clock, wait_clock):
        drain_inst = self.nc.sync.drain()
        wait_clock.add_sem_waits(
            drain_inst.ins, ScopedClock({None: tick_clock.global_clock})
        )
        si = drain_inst.ins.sync_info
        waits = list(si.on_wait) if si is not None else []
        if len(waits) > 1:
            si.on_wait = waits[:1]
            for w in waits[1:]:
                extra = self.nc.sync.drain()
                esi = extra.ins.sync_info
                if esi is None:
                    extra.ins.sync_info = mybir.SyncInfo(on_wait=[w], on_update=[])
                else:
                    esi.on_wait = [w]
        self.nc.all_engine_barrier()
        assert self.sems is not None
        popped = self.nc._tile_sem_poison_stack.pop()
        assert popped is self._sem_poison
        self.nc.clear_and_free_semaphores(list(self.sems.allocated().values()))
        self.nc.all_engine_barrier()

    tile.TileContext._drain_and_barrier = _split_drain_and_barrier
    tile.TileContext._drain_patch_installed = True


_PROGRAM_CACHE: dict = {}


def _build_program(idx):
    """Build the per-core Bass program. `idx` (25 sorted feature columns)
    are compile-time constants, so every DMA is a static access pattern."""
    import concourse.bass as bass
    import concourse.tile as tile
    import concourse.mybir as mybir

    _install_drain_patch()

    f32 = mybir.dt.float32
    nc = bass.Bass()
    xt = nc.dram_tensor("xt", [IN_DIM, BPC], f32, kind="ExternalInput")
    d = nc.dram_tensor("d", [BPC, OUT_DIM], f32, kind="ExternalOutput")
    dfull = nc.dram_tensor("dfull", [BPC, IN_DIM], f32, kind="ExternalOutput")

    with tile.TileContext(nc) as tc:
        with tc.tile_pool(name="z", bufs=1) as zp:
            Z = zp.tile([128, CHUNK], f32)
            G = zp.tile([BPC, OUT_DIM], f32)
            nc.vector.memset(Z[:], 0.0)

            for o, f in enumerate(idx):
                c, j = divmod(f, CHUNK)
                col = xt[f : f + 1, 0:BPC].rearrange("a b -> b a")  # (BPC, 1)
                # poke the live column into the d_full image
                nc.sync.dma_start(out=Z[c * BPC : (c + 1) * BPC, j : j + 1], in_=col)
                # and into the compacted d tile
                nc.sync.dma_start(out=G[0:BPC, o : o + 1], in_=col)

            for c in range(NCHUNK):
                width = CHUNK if c < NCHUNK - 1 else LAST
                nc.sync.dma_start(
                    out=dfull[:, c * CHUNK : c * CHUNK + width],
                    in_=Z[c * BPC : (c + 1) * BPC, 0:width],
                )
            nc.sync.dma_start(out=d[:], in_=G[:])

    return nc


def _derive_structure(measure, fullmeasure):
    """Extract the 25 selected columns; verify the structural assumptions
    the kernel relies on. Returns None if they don't hold."""
    measure = np.asarray(measure, dtype=np.float32)
    if measure.shape != (OUT_DIM, IN_DIM):
        return None
    idx = measure.argmax(axis=1)
    onehot = np.zeros_like(measure)
    onehot[np.arange(OUT_DIM), idx] = 1.0
    if not np.array_equal(measure, onehot):
        return None
    diag = np.asarray(np.diagonal(fullmeasure), dtype=np.float32)
    expect = np.zeros(IN_DIM, dtype=np.float32)
    expect[idx] = 1.0
    if not np.array_equal(diag, expect):
        return None
    return tuple(int(f) for f in idx)


def kernel(input, measure, fullmeasure):
    from concourse.bass_utils import run_bass_kernel_spmd

    x = np.ascontiguousarray(np.asarray(input, dtype=np.float32))
    assert x.shape == (BS, IN_DIM), x.shape

    idx = _derive_structure(measure, fullmeasure)
    if idx is None:
        # Structure violated (cannot happen with the Metric construction):
        # fall back to the dense definition so the result is still correct.
        m = np.asarray(measure, dtype=np.float32)
        fm = np.asarray(fullmeasure, dtype=np.float32)
        return (x @ m.T, x @ fm.T)

    if idx not in _PROGRAM_CACHE:
        _PROGRAM_CACHE[idx] = _build_program(idx)
    nc = _PROGRAM_CACHE[idx]

    in_maps = [
        {"xt": np.ascontiguousarray(x[k * BPC : (k + 1) * BPC, :].T)}
        for k in range(N_CORES)
    ]
    res = run_bass_kernel_spmd(nc, in_maps, core_ids=list(range(N_CORES)))
    d = np.concatenate([res.results[k]["d"] for k in range(N_CORES)], axis=0)
    d_full = np.concatenate([res.results[k]["dfull"] for k in range(N_CORES)], axis=0)
    return (d, d_full)


# revision 8
# speedup vs baseline: 11672.6587x; 11672.6587x over previous
"""Trainium2 Bass kernel for nn_Metric_35588099015209.

Reference computation (BS=64, IN_DIM=9801, OUT_DIM=25):
    d      = einsum('oi,bi->bo', measure,     input)   # (64, 25)
    d_full = einsum('fi,bi->bf', fullmeasure, input)   # (64, 9801)

Structure exploited (guaranteed by the Metric module's construction and
verified at runtime before use): `measure` is 25 distinct rows of the
identity, and `fullmeasure` is the identity with the complementary rows
zeroed — a diagonal 0/1 matrix whose 25 ones sit exactly at the columns
`measure` selects. Therefore:
    d[b, o]      = input[b, idx[o]]               (a 25-column gather)
    d_full[b, f] = input[b, f] * mask[f]          (an elementwise mask)

Sharding: data-parallel over the batch — each of the 8 NeuronCores gets 8
rows of `input` and produces its 8 rows of both outputs; the mask / index
constants are replicated. No communication.

Per-core kernel (raw Bass, explicit single-wait semaphores — this
toolchain's walrus rejects instructions carrying more than one sem-wait;
DMA-issue cost and DMA-completion latency dominate at this size, so the
design minimizes dma_start count; 9801 = 11*891 gives an exact 2D SBUF
layout with partition = chunk*8 + batch_row and a single 3D-access-pattern
store for the whole d_full image):
  SP  : load xz (88,891); after mul half0: store columns [0,445) of every
        chunk; after gather: store d; final completion wait
  Act : load idx table; load mz mask image; after mul half1: store columns
        [445,891)
  DVE : elementwise t_xz *= t_mz in two column halves (store of half 0
        overlaps multiply of half 1)
  Pool: indirect-DMA gather of the 25 live rows of xt (9801,8) -> d tile
"""

import numpy as np

IN_DIM = 9801
OUT_DIM = 25
BS = 64
N_CORES = 8
BPC = BS // N_CORES  # batch rows per core
NCHUNK = 11
CHUNK = 891  # 11 * 891 = 9801 exactly
NPART = NCHUNK * BPC  # 88 partitions


def _build_program():
    import concourse.bass as bass
    import concourse.mybir as mybir
    from contextlib import ExitStack

    f32 = mybir.dt.float32
    i32 = mybir.dt.int32
    nc = bass.Bass()
    xz = nc.dram_tensor("xz", [NPART, CHUNK], f32, kind="ExternalInput")
    mz = nc.dram_tensor("mz", [NPART, CHUNK], f32, kind="ExternalInput")
    xt = nc.dram_tensor("xt", [IN_DIM, BPC], f32, kind="ExternalInput")
    idxt = nc.dram_tensor("idx", [OUT_DIM, 1], i32, kind="ExternalInput")
    d = nc.dram_tensor("d", [OUT_DIM, BPC], f32, kind="ExternalOutput")
    dfull = nc.dram_tensor("dfull", [BPC, IN_DIM], f32, kind="ExternalOutput")

    with ExitStack() as stack:
        t_idx = stack.enter_context(nc.sbuf_tensor([OUT_DIM, 1], i32))
        t_g = stack.enter_context(nc.sbuf_tensor([OUT_DIM, BPC], f32))
        t_xz = stack.enter_context(nc.sbuf_tensor([NPART, CHUNK], f32))
        t_mz = stack.enter_context(nc.sbuf_tensor([NPART, CHUNK], f32))
        s_idx = stack.enter_context(nc.semaphore("s_idx"))
        s_g = stack.enter_context(nc.semaphore("s_g"))
        s_fin = stack.enter_context(nc.semaphore("s_fin"))
        s_ld = stack.enter_context(nc.semaphore("s_ld"))
        s_mul = stack.enter_context(nc.semaphore("s_mul"))
        block = stack.enter_context(nc.Block(no_gpsimd_drain=True))

        J0 = CHUNK // 2  # multiply/store split point along the free dim
        # d_full viewed as (chunk, batch, j): matches t_xz's element order
        # (the SBUF side must stay a plain 2D AP — split-partition SBUF APs
        # mis-lower in this toolchain).
        dram3d = dfull[:, :].rearrange("b (c j) -> c b j", j=CHUNK)

        @block.sync
        def _(sync):
            sync.dma_start(out=t_xz[:], in_=xz[:]).then_inc(s_ld, 16)
            sync.wait_ge(s_mul, 1)
            sync.dma_start(out=dram3d[:, :, :J0], in_=t_xz[:, :J0]).then_inc(
                s_fin, 16
            )
            sync.wait_ge(s_g, 16)
            sync.dma_start(out=d[:], in_=t_g[:]).then_inc(s_fin, 16)
            sync.wait_ge(s_fin, 48)

        @block.scalar
        def _(scalar):
            scalar.dma_start(out=t_idx[:], in_=idxt[:]).then_inc(s_idx, 16)
            scalar.dma_start(out=t_mz[:], in_=mz[:]).then_inc(s_ld, 16)
            scalar.wait_ge(s_mul, 2)
            scalar.dma_start(out=dram3d[:, :, J0:], in_=t_xz[:, J0:]).then_inc(
                s_fin, 16
            )

        @block.vector
        def _(vector):
            vector.wait_ge(s_ld, 32)
            vector.tensor_mul(
                out=t_xz[:, :J0], in0=t_xz[:, :J0], in1=t_mz[:, :J0]
            ).then_inc(s_mul, 1)
            vector.tensor_mul(
                out=t_xz[:, J0:], in0=t_xz[:, J0:], in1=t_mz[:, J0:]
            ).then_inc(s_mul, 1)

        @block.gpsimd
        def _(gpsimd):
            gpsimd.wait_ge(s_idx, 16)
            gpsimd.indirect_dma_start(
                out=t_g[:],
                out_offset=None,
                in_=xt[:],
                in_offset=bass.IndirectOffsetOnAxis(ap=t_idx[:, :1], axis=0),
            ).then_inc(s_g, 16)

    return nc


_PROGRAM_CACHE: dict = {}


def _derive_structure(measure, fullmeasure):
    """Extract the 25 selected columns; verify the structural assumptions
    the kernel relies on. Returns None if they don't hold."""
    measure = np.asarray(measure, dtype=np.float32)
    if measure.shape != (OUT_DIM, IN_DIM):
        return None
    idx = measure.argmax(axis=1)
    onehot = np.zeros_like(measure)
    onehot[np.arange(OUT_DIM), idx] = 1.0
    if not np.array_equal(measure, onehot):
        return None
    diag = np.asarray(np.diagonal(fullmeasure), dtype=np.float32)
    expect = np.zeros(IN_DIM, dtype=np.float32)
    expect[idx] = 1.0
    if not np.array_equal(diag, expect):
        return None
    return tuple(int(f) for f in idx)


def kernel(input, measure, fullmeasure):
    from concourse.bass_utils import run_bass_kernel_spmd

    x = np.ascontiguousarray(np.asarray(input, dtype=np.float32))
    assert x.shape == (BS, IN_DIM), x.shape

    idx = _derive_structure(measure, fullmeasure)
    if idx is None:
        # Structure violated (cannot happen with the Metric construction):
        # fall back to the dense definition so the result is still correct.
        m = np.asarray(measure, dtype=np.float32)
        fm = np.asarray(fullmeasure, dtype=np.float32)
        return (x @ m.T, x @ fm.T)

    if "nc" not in _PROGRAM_CACHE:
        _PROGRAM_CACHE["nc"] = _build_program()
    nc = _PROGRAM_CACHE["nc"]

    mask = np.zeros(IN_DIM, dtype=np.float32)
    mask[list(idx)] = 1.0
    # mask in the (chunk*BPC + b, j) SBUF layout; identical for every b
    mz = np.ascontiguousarray(
        np.broadcast_to(
            mask.reshape(NCHUNK, 1, CHUNK), (NCHUNK, BPC, CHUNK)
        ).reshape(NPART, CHUNK)
    )
    idx_arr = np.asarray(idx, dtype=np.int32).reshape(OUT_DIM, 1)

    in_maps = []
    for k in range(N_CORES):
        shard = x[k * BPC : (k + 1) * BPC, :]  # (8, 9801)
        xz = np.ascontiguousarray(
            shard.reshape(BPC, NCHUNK, CHUNK).transpose(1, 0, 2).reshape(NPART, CHUNK)
        )
        xt = np.ascontiguousarray(shard.T)
        in_maps.append({"xz": xz, "mz": mz, "xt": xt, "idx": idx_arr})

    res = run_bass_kernel_spmd(nc, in_maps, core_ids=list(range(N_CORES)))
    d = np.concatenate([res.results[k]["d"].T for k in range(N_CORES)], axis=0)
    d_full = np.concatenate(
        [res.results[k]["dfull"] for k in range(N_CORES)], axis=0
    )
    return (d, d_full)
